# revision 9
# baseline (speedup 1.0000x reference)
"""Distributed Trainium2 Bass kernel for the GAT-Actor (gnn_message_passing).

Strategy (8 NeuronCores, 1-D node partition):
  - nodes sharded contiguously: core i owns rows [i*NLOC, (i+1)*NLOC)
  - edges assigned to the core owning their DESTINATION node
  - stage 1 (f32): h = x_shard @ W (x pre-transposed on host); per node a
    768B f32 table row [h(128) | 1.0 | e_src | pad]; rows are written in two
    pieces (locals < 3200 / >= 3200) and AllGathered piece-wise so the edge
    stage can start after the first collective.
  - stage 2: edges sorted by (dst-chunk, src-piece) into 128-edge blocks;
    dma_gather pulls 768B rows.  The dst-scatter onehot P[e, d] per block is
    HOST-precomputed (0/1 bf16, pad rows zero) and streamed from HBM - no
    on-device onehot builds.  Per block:
      tloc (e_dst per edge) = DVE stt accum of P * broadcast(e_dst row),
      S = ACT copy-scale of P by w (per-partition scalar, f32 out),
      agg[d, 0:130] += S^T [h | 1 | e_src]  (TensorE; col 128 = softmax
      denominator - ones column fused, no separate den matmul).
    Chunk tail: scale rows by 1/den, PE-transpose to feature-major,
    bias+relu -> h0T f32.
  - stage 3 (f32): BN stats via 1KB AllReduce folded into rescaled fc
    weights; fc1/fc2/fc3 on TensorE; row softmax; [NLOC, 32] shards
    concatenated on host.
"""

import os
import sys

for _p in ("/opt/trn_rl_repo", "/root/.axon_site/_ro/trn_rl_repo"):
    if os.path.isdir(_p) and _p not in sys.path:
        sys.path.insert(0, _p)

import numpy as np
import ml_dtypes

from concourse import bass, bacc, tile, mybir
from concourse.bass_utils import run_bass_kernel_spmd

f32 = mybir.dt.float32
bf16 = mybir.dt.bfloat16
i16 = mybir.dt.int16
AF = mybir.ActivationFunctionType
ALU = mybir.AluOpType

NCORES = 8
C = 128                # dst-chunk width
NEG_SLOPE = 0.2
EPS = 1e-5
PL = 3200              # piece boundary in local rows (25 tiles of 128)
G_CH = 2               # chunks per gather group

_cache = {}
last_results = None


# --------------------------------------------------------------------------
# host-side edge preprocessing
# --------------------------------------------------------------------------

def _wrap_idx(idx):
    """int16 index stream -> [128, len/16] wrapped+replicated for dma_gather."""
    idx = np.asarray(idx, np.int16)
    m = idx.shape[0]
    assert m % 16 == 0
    arr = idx.reshape(m // 16, 16).T
    return np.ascontiguousarray(np.tile(arr, (8, 1)))


def _prep_edges(edge_index, N, NLOC):
    """Sort edges per dst-core by (dst-chunk, src-piece); pad each
    (chunk, piece) to 128-edge blocks shared across cores.  Returns the
    per-core index streams + onehot P tiles and the shared block layout.
    """
    src = np.asarray(edge_index[0], np.int64)
    dst = np.asarray(edge_index[1], np.int64)
    NCH = -(-NLOC // C)
    PB = NLOC - PL

    cores = []
    counts = np.zeros((NCORES, NCH, 2), np.int64)
    for i in range(NCORES):
        sel = (dst // NLOC) == i
        s = src[sel]
        d = dst[sel] - i * NLOC
        ch = d // C
        cs = s // NLOC
        loc = s % NLOC
        hf = (loc >= PL).astype(np.int64)
        idx16 = np.where(hf == 0, cs * PL + loc, cs * PB + (loc - PL))
        order = np.lexsort((hf, ch))
        s_i, d_i, ch_i, hf_i = idx16[order], d[order], ch[order], hf[order]
        for c in range(NCH):
            m = ch_i == c
            counts[i, c, 0] = np.count_nonzero(m & (hf_i == 0))
            counts[i, c, 1] = np.count_nonzero(m & (hf_i == 1))
        cores.append((s_i, d_i, ch_i, hf_i))

    NA = [int(-(-counts[:, c, 0].max() // 128)) for c in range(NCH)]
    NB = [int(-(-counts[:, c, 1].max() // 128)) for c in range(NCH)]

    groups = [list(range(g, min(g + G_CH, NCH))) for g in range(0, NCH, G_CH)]

    blk_of = {}
    goff = 0
    ginfo = []
    for chunks in groups:
        nA = sum(NA[c] for c in chunks)
        nB = sum(NB[c] for c in chunks)
        off = goff
        for c in chunks:
            blk_of[(c, 0)] = off
            off += NA[c]
        for c in chunks:
            blk_of[(c, 1)] = off
            off += NB[c]
        ginfo.append((chunks, goff, nA, nB))
        goff += nA + nB
    TOTB = goff
    TOTE = TOTB * 128

    dr = np.arange(C, dtype=np.int64)
    per_core = []
    for i in range(NCORES):
        s_i, d_i, ch_i, hf_i = cores[i]
        src_idx = np.zeros(TOTE, np.int16)
        dst_rel = np.full(TOTE, -1, np.int64)
        ptr = 0
        for c in range(NCH):
            for h in (0, 1):
                cnt = int(counts[i, c, h])
                sl = slice(ptr, ptr + cnt)
                ptr += cnt
                pos = blk_of[(c, h)] * 128
                if cnt:
                    src_idx[pos:pos + cnt] = s_i[sl].astype(np.int16)
                    dst_rel[pos:pos + cnt] = d_i[sl] - c * C
        assert ptr == len(s_i)
        # P tiles: [TOTB*128, 128] bf16, row b*128+e = onehot(dst_rel)
        drel = dst_rel.reshape(TOTB, 128)
        P = (drel[:, :, None] == dr[None, None, :]).astype(ml_dtypes.bfloat16)
        per_core.append({
            "src_idx": _wrap_idx(src_idx),
            "P": np.ascontiguousarray(P.reshape(TOTB * 128, 128)),
        })
    return per_core, NA, NB, ginfo, blk_of, TOTB


# --------------------------------------------------------------------------
# device graph
# --------------------------------------------------------------------------

def _build_nc(N, D, H, A, NLOC, NA, NB, ginfo, blk_of, TOTB):
    KD = D // 128
    NT = -(-NLOC // 128)
    NLOCP = NT * 128
    NCH = len(NA)
    PB = NLOC - PL
    RA = NCORES * PL
    RB = NCORES * PB
    ROWW = 192                # f32 elems per table row (768B)
    MRW = 130                 # meaningful row width: h(128) | 1 | e_src

    nc = bacc.Bacc("TRN2", num_devices=NCORES)

    xT_in = nc.dram_tensor("xT_shard", [D, NLOC], f32, kind="ExternalInput")
    W_in = nc.dram_tensor("W", [D, H], f32, kind="ExternalInput")
    asrcb = nc.dram_tensor("asrc_b", [128, H], f32, kind="ExternalInput")
    adstb = nc.dram_tensor("adst_b", [128, H], f32, kind="ExternalInput")
    bgat = nc.dram_tensor("b_gat", [H, 1], f32, kind="ExternalInput")
    bn0p = nc.dram_tensor("bn0p", [H, 2], f32, kind="ExternalInput")
    bn2p = nc.dram_tensor("bn2p", [H, 2], f32, kind="ExternalInput")
    W1_in = nc.dram_tensor("W1", [H, H], f32, kind="ExternalInput")
    b1_in = nc.dram_tensor("b1", [H, 1], f32, kind="ExternalInput")
    W2_in = nc.dram_tensor("W2", [H, H], f32, kind="ExternalInput")
    b2_in = nc.dram_tensor("b2", [H, 1], f32, kind="ExternalInput")
    W3_in = nc.dram_tensor("W3", [H, A], f32, kind="ExternalInput")
    b3_in = nc.dram_tensor("b3", [A, 1], f32, kind="ExternalInput")
    ident_in = nc.dram_tensor("ident", [128, 128], f32, kind="ExternalInput")
    onesrow_in = nc.dram_tensor("ones_row", [1, 128], f32, kind="ExternalInput")
    srci_in = nc.dram_tensor("src_idx", [128, TOTB * 8], i16, kind="ExternalInput")
    P_in = nc.dram_tensor("P", [TOTB * 128, 128], bf16, kind="ExternalInput")

    out_t = nc.dram_tensor("out", [NLOC, A], f32, kind="ExternalOutput")

    with tile.TileContext(nc) as tc:
        with tc.tile_pool(name="const", bufs=1) as cp, \
             tc.tile_pool(name="dram", bufs=1, space="DRAM") as dram, \
             tc.tile_pool(name="big", bufs=1) as bigp:

            srci_sb = bigp.tile([128, TOTB * 8], i16)
            nc.sync.dma_start(srci_sb[:], srci_in[:])
            W_sb = cp.tile([128, KD, H], f32)
            nc.sync.dma_start(W_sb[:], bass.AP(W_in, 0, [[H, 128], [128 * H, KD], [1, H]]))
            ident = cp.tile([128, 128], f32)
            nc.sync.dma_start(ident[:], ident_in[:])
            asrc_sb = cp.tile([128, H], f32)
            nc.sync.dma_start(asrc_sb[:], asrcb[:])
            adst_sb = cp.tile([128, H], f32)
            nc.sync.dma_start(adst_sb[:], adstb[:])
            bgat_sb = cp.tile([H, 1], f32)
            nc.sync.dma_start(bgat_sb[:], bgat[:])
            bn0_sb = cp.tile([H, 2], f32)
            nc.sync.dma_start(bn0_sb[:], bn0p[:])
            bn2_sb = cp.tile([H, 2], f32)
            nc.sync.dma_start(bn2_sb[:], bn2p[:])
            W1_sb = cp.tile([H, H], f32)
            nc.sync.dma_start(W1_sb[:], W1_in[:])
            b1_sb = cp.tile([H, 1], f32)
            nc.sync.dma_start(b1_sb[:], b1_in[:])
            W2_sb = cp.tile([H, H], f32)
            nc.sync.dma_start(W2_sb[:], W2_in[:])
            b2_sb = cp.tile([H, 1], f32)
            nc.sync.dma_start(b2_sb[:], b2_in[:])
            W3_sb = cp.tile([H, A], f32)
            nc.sync.dma_start(W3_sb[:], W3_in[:])
            b3_sb = cp.tile([A, 1], f32)
            nc.sync.dma_start(b3_sb[:], b3_in[:])
            onesr = cp.tile([1, 128], f32)
            nc.sync.dma_start(onesr[:], onesrow_in[:])

            hlocA = dram.tile([PL, ROWW], f32)
            hlocB = dram.tile([PB, ROWW], f32)
            hfullA = dram.tile([RA, ROWW], f32, addr_space="Shared")
            hfullB = dram.tile([RB, ROWW], f32, addr_space="Shared")
            bn_in_0 = dram.tile([H, 2], f32)
            bn_out_0 = dram.tile([H, 2], f32, addr_space="Shared")
            bn_in_1 = dram.tile([H, 2], f32)
            bn_out_1 = dram.tile([H, 2], f32, addr_space="Shared")

            edstloc = bigp.tile([128, NT], f32)

            # ================= stage 1: h rows + e_src/e_dst ================
            with tc.tile_pool(name="s1", bufs=3) as s1p, \
                 tc.tile_pool(name="s1ps", bufs=2, space="PSUM") as s1ps:
                for t in range(NT):
                    rows = min(128, NLOC - t * 128)
                    xT_t = s1p.tile([128, KD, 128], f32, tag="xt")
                    for k in range(KD):
                        nc.sync.dma_start(
                            xT_t[:, k, 0:rows],
                            xT_in[k * 128:(k + 1) * 128,
                                  t * 128:t * 128 + rows])
                    h_ps = s1ps.tile([128, H], f32, tag="hps")
                    for k in range(KD):
                        nc.tensor.matmul(h_ps[:], xT_t[:, k, :], W_sb[:, k, :],
                                         start=(k == 0), stop=(k == KD - 1))
                    h_row = s1p.tile([128, MRW], f32, tag="hrow")
                    nc.vector.tensor_copy(h_row[:, 0:H], h_ps[:])
                    nc.vector.memset(h_row[:, H:H + 1], 1.0)
                    scr = s1p.tile([128, H], f32, tag="scr")
                    nc.vector.scalar_tensor_tensor(
                        out=scr[:], in0=h_ps[:], scalar=1.0, in1=asrc_sb[:],
                        op0=ALU.mult, op1=ALU.mult,
                        accum_out=h_row[:, MRW - 1:MRW])
                    scr2 = s1p.tile([128, H], f32, tag="scr2")
                    nc.vector.scalar_tensor_tensor(
                        out=scr2[:], in0=h_ps[:], scalar=1.0, in1=adst_sb[:],
                        op0=ALU.mult, op1=ALU.mult,
                        accum_out=edstloc[:, t:t + 1])
                    if t < 25:
                        nc.sync.dma_start(
                            bass.AP(hlocA.tensor, t * 128 * ROWW,
                                    [[ROWW, rows], [1, MRW]]),
                            h_row[:rows, :])
                    else:
                        r0 = (t - 25) * 128
                        nc.sync.dma_start(
                            bass.AP(hlocB.tensor, r0 * ROWW,
                                    [[ROWW, rows], [1, MRW]]),
                            h_row[:rows, :])
                    if t == 24:
                        # piece A complete: AllGather it while piece B computes
                        nc.gpsimd.collective_compute(
                            "AllGather", ALU.bypass,
                            replica_groups=[list(range(NCORES))],
                            ins=[hlocA.opt()], outs=[hfullA.opt()])

            nc.gpsimd.collective_compute(
                "AllGather", ALU.bypass, replica_groups=[list(range(NCORES))],
                ins=[hlocB.opt()], outs=[hfullB.opt()])

            # ================= stage 2: edge aggregation ===================
            h0T = bigp.tile([128, NLOCP], f32)
            if NLOC != NLOCP:
                nc.vector.memset(h0T[:, NLOC:NLOCP], 0.0)
            s1cols = bigp.tile([128, NCH], f32)
            s2cols = bigp.tile([128, NCH], f32)
            with tc.tile_pool(name="s2", bufs=2) as s2p, \
                 tc.tile_pool(name="s2s", bufs=4) as s2s, \
                 tc.tile_pool(name="s2ps", bufs=2, space="PSUM") as s2ps:
                for chunks, goff, nAg, nBg in ginfo:
                    nblk = nAg + nBg
                    g_t = s2p.tile([128, nblk, ROWW], f32, tag="g")
                    if nAg:
                        nc.gpsimd.dma_gather(
                            g_t[:, 0:nAg, :], hfullA[:],
                            srci_sb[:, goff * 8: (goff + nAg) * 8],
                            nAg * 128, nAg * 128, ROWW, single_packet=False)
                    if nBg:
                        nc.gpsimd.dma_gather(
                            g_t[:, nAg:nblk, :], hfullB[:],
                            srci_sb[:, (goff + nAg) * 8: (goff + nblk) * 8],
                            nBg * 128, nBg * 128, ROWW, single_packet=False)
                    P_t = s2p.tile([128, nblk, 128], bf16, tag="P")
                    nc.sync.dma_start(
                        P_t[:],
                        bass.AP(P_in, goff * 128 * 128,
                                [[128, 128], [128 * 128, nblk], [1, 128]]))

                    for c in chunks:
                        na, nb = NA[c], NB[c]
                        nbf = na + nb
                        aoff = blk_of[(c, 0)] - goff
                        boff = blk_of[(c, 1)] - goff
                        Cc = min(C, NLOC - c * C)
                        blist = list(range(aoff, aoff + na)) + \
                                list(range(boff, boff + nb))

                        # e_dst broadcast row for this chunk
                        edT_ps = s2ps.tile([1, 128], f32, tag="edT", bufs=2)
                        nc.tensor.matmul(edT_ps[:], edstloc[:, c:c + 1],
                                         ident[:], start=True, stop=True)
                        edrow = s2p.tile([1, 128], f32, tag="edrow", bufs=2)
                        nc.vector.tensor_copy(edrow[:], edT_ps[:])
                        edB_ps = s2ps.tile([128, 128], f32, tag="edB", bufs=2)
                        nc.tensor.matmul(edB_ps[:], onesr[:], edrow[:],
                                         start=True, stop=True)
                        edb = s2p.tile([128, 128], f32, tag="edb", bufs=2)
                        nc.vector.tensor_copy(edb[:], edB_ps[:])

                        # per-edge e_dst: tloc[e] = sum_d P[e,d] * edb[.,d]
                        tlocv = s2p.tile([128, nbf], f32, tag="tloc", bufs=2)
                        for j, b in enumerate(blist):
                            scrT = s2s.tile([128, C], f32, tag="scrT", bufs=8)
                            nc.vector.scalar_tensor_tensor(
                                out=scrT[:], in0=P_t[:, b, :], scalar=1.0,
                                in1=edb[:], op0=ALU.mult, op1=ALU.mult,
                                accum_out=tlocv[:, j:j + 1])

                        # w = exp(leaky(e_src + e_dst))
                        eps_t = s2p.tile([128, nbf], f32, tag="eps", bufs=2)
                        nc.vector.tensor_tensor(
                            out=eps_t[:, 0:na],
                            in0=g_t[:, aoff:aoff + na, MRW - 1],
                            in1=tlocv[:, 0:na], op=ALU.add)
                        nc.vector.tensor_tensor(
                            out=eps_t[:, na:nbf],
                            in0=g_t[:, boff:boff + nb, MRW - 1],
                            in1=tlocv[:, na:nbf], op=ALU.add)
                        lk = s2p.tile([128, nbf], f32, tag="lk", bufs=2)
                        nc.vector.scalar_tensor_tensor(
                            out=lk[:], in0=eps_t[:], scalar=NEG_SLOPE,
                            in1=eps_t[:], op0=ALU.mult, op1=ALU.max)
                        w_t = s2p.tile([128, nbf], f32, tag="w", bufs=2)
                        nc.scalar.activation(w_t[:], lk[:], AF.Exp)

                        # agg[d, 0:130] += S^T [h | 1 | e_src],  S = P * w
                        agg_ps = s2ps.tile([128, MRW], f32, tag="agg", bufs=2)
                        for j, b in enumerate(blist):
                            S_b = s2s.tile([128, C], f32, tag="S", bufs=8)
                            nc.scalar.activation(S_b[:], P_t[:, b, :], AF.Copy,
                                                 scale=w_t[:, j:j + 1])
                            nc.tensor.matmul(agg_ps[:], S_b[:],
                                             g_t[:, b, 0:MRW],
                                             start=(j == 0), stop=(j == nbf - 1))

                        den = s2p.tile([128, 1], f32, tag="den", bufs=2)
                        nc.vector.tensor_scalar(
                            out=den[:], in0=agg_ps[:, H:H + 1],
                            scalar1=1e-16, scalar2=None, op0=ALU.max)
                        rden = s2p.tile([128, 1], f32, tag="rden", bufs=2)
                        nc.vector.reciprocal(rden[:], den[:])
                        h0n = s2p.tile([128, 128], f32, tag="h0n", bufs=2)
                        nc.vector.tensor_scalar(
                            out=h0n[:], in0=agg_ps[:, 0:H],
                            scalar1=rden[:], scalar2=None, op0=ALU.mult)
                        tr_ps = s2ps.tile([128, 128], f32, tag="tr", bufs=2)
                        nc.tensor.transpose(tr_ps[:], h0n[:], ident[:])
                        nc.vector.tensor_scalar(
                            out=h0T[:, c * C: c * C + Cc], in0=tr_ps[:, 0:Cc],
                            scalar1=bgat_sb[:], scalar2=0.0,
                            op0=ALU.add, op1=ALU.max)
                        # incremental BN0 stats for this chunk
                        nc.vector.tensor_reduce(
                            out=s1cols[:, c:c + 1],
                            in_=h0T[:, c * C: c * C + Cc],
                            axis=mybir.AxisListType.X, op=ALU.add)
                        sqv = s2s.tile([128, C], f32, tag="sqv", bufs=4)
                        nc.vector.scalar_tensor_tensor(
                            out=sqv[:, 0:Cc], in0=h0T[:, c * C: c * C + Cc],
                            scalar=1.0, in1=h0T[:, c * C: c * C + Cc],
                            op0=ALU.mult, op1=ALU.mult,
                            accum_out=s2cols[:, c:c + 1])

            # ================= stage 3: BN0 + MLP + softmax ================
            with tc.tile_pool(name="s3", bufs=2) as s3p, \
                 tc.tile_pool(name="s3ps", bufs=2, space="PSUM") as s3ps:

                def bn_fold(hT, k, Wnext_sb, bnext_sb, M, stats=None):
                    s1 = s3p.tile([128, 1], f32, tag="bn1")
                    s2 = s3p.tile([128, 1], f32, tag="bn2t")
                    if stats is not None:
                        nc.vector.tensor_reduce(out=s1[:], in_=stats[0][:],
                                                axis=mybir.AxisListType.X,
                                                op=ALU.add)
                        nc.vector.tensor_reduce(out=s2[:], in_=stats[1][:],
                                                axis=mybir.AxisListType.X,
                                                op=ALU.add)
                    else:
                        nc.vector.tensor_reduce(out=s1[:], in_=hT[:, 0:NLOC],
                                                axis=mybir.AxisListType.X,
                                                op=ALU.add)
                        nsq = -(-NLOC // 512)
                        sqcols = s3p.tile([128, nsq], f32, tag="bnsq" + str(k))
                        for si in range(nsq):
                            s0 = si * 512
                            ln = min(512, NLOC - s0)
                            sq = s3p.tile([128, 512], f32, tag="sqscr", bufs=2)
                            nc.scalar.activation(sq[:, 0:ln], hT[:, s0:s0 + ln],
                                                 AF.Square,
                                                 accum_out=sqcols[:, si:si + 1])
                        nc.vector.tensor_reduce(out=s2[:], in_=sqcols[:],
                                                axis=mybir.AxisListType.X,
                                                op=ALU.add)
                    bnio = s3p.tile([128, 2], f32, tag="bnio")
                    nc.vector.tensor_copy(bnio[:, 0:1], s1[:])
                    nc.vector.tensor_copy(bnio[:, 1:2], s2[:])
                    bn_in_d = bn_in_0 if k == 0 else bn_in_1
                    bn_out_d = bn_out_0 if k == 0 else bn_out_1
                    nc.sync.dma_start(bn_in_d[:], bnio[:])
                    nc.gpsimd.collective_compute(
                        "AllReduce", ALU.add, replica_groups=[list(range(NCORES))],
                        ins=[bn_in_d.opt()], outs=[bn_out_d.opt()])
                    bnst = s3p.tile([128, 2], f32, tag="bnst")
                    nc.sync.dma_start(bnst[:], bn_out_d[:])
                    mu = s3p.tile([128, 1], f32, tag="mu")
                    nc.vector.tensor_scalar(out=mu[:], in0=bnst[:, 0:1],
                                            scalar1=1.0 / N, scalar2=None,
                                            op0=ALU.mult)
                    var = s3p.tile([128, 1], f32, tag="var")
                    nc.vector.tensor_tensor(out=var[:], in0=mu[:], in1=mu[:],
                                            op=ALU.mult)
                    nc.vector.tensor_scalar(out=var[:], in0=var[:], scalar1=-1.0,
                                            scalar2=None, op0=ALU.mult)
                    nc.vector.scalar_tensor_tensor(
                        out=var[:], in0=bnst[:, 1:2], scalar=1.0 / N, in1=var[:],
                        op0=ALU.mult, op1=ALU.add)
                    nc.vector.tensor_scalar(out=var[:], in0=var[:], scalar1=EPS,
                                            scalar2=None, op0=ALU.add)
                    rs = s3p.tile([128, 1], f32, tag="rs")
                    nc.vector.reciprocal(rs[:], var[:])
                    nc.scalar.sqrt(rs[:], rs[:])
                    bnp = bn0_sb if k == 0 else bn2_sb
                    sc = s3p.tile([128, 1], f32, tag="sc")
                    nc.vector.tensor_tensor(out=sc[:], in0=rs[:], in1=bnp[:, 0:1],
                                            op=ALU.mult)
                    u = s3p.tile([128, 1], f32, tag="u")
                    nc.vector.tensor_tensor(out=u[:], in0=mu[:], in1=sc[:],
                                            op=ALU.mult)
                    nc.vector.tensor_sub(u[:], bnp[:, 1:2], u[:])
                    Wp = s3p.tile([128, M], f32, tag="wp" + str(k))
                    nc.vector.tensor_scalar(out=Wp[:], in0=Wnext_sb[:],
                                            scalar1=sc[:], scalar2=None,
                                            op0=ALU.mult)
                    brow_ps = s3ps.tile([1, M], f32, tag="brow", bufs=1)
                    nc.tensor.matmul(brow_ps[:], u[:], Wnext_sb[:],
                                     start=True, stop=True)
                    brow_sb = s3p.tile([1, M], f32, tag="brsb")
                    nc.vector.tensor_copy(brow_sb[:], brow_ps[:])
                    bcol_ps = s3ps.tile([M, 1], f32, tag="bcol", bufs=1)
                    nc.tensor.transpose(bcol_ps[:], brow_sb[:], ident[0:1, 0:1])
                    bp = s3p.tile([M, 1], f32, tag="bp" + str(k))
                    nc.vector.tensor_tensor(out=bp[:], in0=bcol_ps[:],
                                            in1=bnext_sb[:], op=ALU.add)
                    return Wp, bp

                h1T = bigp.tile([128, NLOCP], f32)
                W1p, b1p = bn_fold(h0T, 0, W1_sb, b1_sb, H,
                                   stats=(s1cols, s2cols))
                for s in range(0, NLOC, 512):
                    ln = min(512, NLOC - s)
                    ps = s3ps.tile([128, 512], f32, tag="mlp")
                    nc.tensor.matmul(ps[:, 0:ln], W1p[:], h0T[:, s:s + ln],
                                     start=True, stop=True)
                    nc.scalar.activation(h1T[:, s:s + ln], ps[:, 0:ln], AF.Relu,
                                         bias=b1p[:])
                h2T = h0T  # overwrite in place
                for s in range(0, NLOC, 512):
                    ln = min(512, NLOC - s)
                    ps = s3ps.tile([128, 512], f32, tag="mlp")
                    nc.tensor.matmul(ps[:, 0:ln], W2_sb[:], h1T[:, s:s + ln],
                                     start=True, stop=True)
                    nc.scalar.activation(h2T[:, s:s + ln], ps[:, 0:ln], AF.Relu,
                                         bias=b2_sb[:])
                W3p, b3p = bn_fold(h2T, 1, W3_sb, b3_sb, A)
                actT = bigp.tile([A, NLOCP], f32)
                for s in range(0, NLOC, 512):
                    ln = min(512, NLOC - s)
                    ps = s3ps.tile([A, 512], f32, tag="mlp3")
                    nc.tensor.matmul(ps[:, 0:ln], W3p[:], h2T[:, s:s + ln],
                                     start=True, stop=True)
                    nc.vector.tensor_scalar(out=actT[0:A, s:s + ln],
                                            in0=ps[:, 0:ln],
                                            scalar1=b3p[:], scalar2=None,
                                            op0=ALU.add)
                for t in range(NT):
                    rows = min(128, NLOC - t * 128)
                    a_sb = s3p.tile([128, A], f32, tag="asb")
                    for sub in range(4):
                        nc.vector.transpose(
                            a_sb[32 * sub:32 * sub + 32, 0:A],
                            actT[0:A, t * 128 + 32 * sub: t * 128 + 32 * sub + 32])
                    nmax = s3p.tile([128, 1], f32, tag="nmax")
                    nc.vector.tensor_reduce(out=nmax[:], in_=a_sb[:],
                                            axis=mybir.AxisListType.X, op=ALU.max)
                    nc.vector.tensor_scalar(out=nmax[:], in0=nmax[:],
                                            scalar1=-1.0, scalar2=None,
                                            op0=ALU.mult)
                    e_sb = s3p.tile([128, A], f32, tag="esb")
                    nc.scalar.activation(e_sb[:], a_sb[:], AF.Exp, bias=nmax[:])
                    ssum = s3p.tile([128, 1], f32, tag="ssum")
                    nc.vector.tensor_reduce(out=ssum[:], in_=e_sb[:],
                                            axis=mybir.AxisListType.X, op=ALU.add)
                    rsum = s3p.tile([128, 1], f32, tag="rsum")
                    nc.vector.reciprocal(rsum[:], ssum[:])
                    o_sb = s3p.tile([128, A], f32, tag="osb")
                    nc.vector.tensor_scalar(out=o_sb[:], in0=e_sb[:],
                                            scalar1=rsum[:], scalar2=None,
                                            op0=ALU.mult)
                    nc.sync.dma_start(out_t[t * 128: t * 128 + rows, :],
                                      o_sb[:rows, :])

    nc.compile()
    return nc


# --------------------------------------------------------------------------
# public entry point
# --------------------------------------------------------------------------

def run(inputs, trace=False):
    global last_results
    x = np.asarray(inputs["x"], np.float32)
    edge_index = np.asarray(inputs["edge_index"])
    N, D = x.shape
    H = np.asarray(inputs["W"]).shape[1]
    A = np.asarray(inputs["W3"]).shape[1]
    assert N % NCORES == 0
    NLOC = N // NCORES

    per_core, NA, NB, ginfo, blk_of, TOTB = _prep_edges(edge_index, N, NLOC)

    key = (N, D, H, A, NLOC, tuple(NA), tuple(NB))
    if _cache.get("key") != key:
        _cache["nc"] = _build_nc(N, D, H, A, NLOC, NA, NB, ginfo, blk_of, TOTB)
        _cache["key"] = key
    nc = _cache["nc"]

    g = lambda k: np.ascontiguousarray(np.asarray(inputs[k], np.float32))
    common = {
        "W": g("W"),
        "asrc_b": np.tile(g("a_src")[None, :], (128, 1)),
        "adst_b": np.tile(g("a_dst")[None, :], (128, 1)),
        "b_gat": g("b_gat").reshape(H, 1),
        "bn0p": np.stack([g("g0"), g("beta0")], 1),
        "bn2p": np.stack([g("g2"), g("beta2")], 1),
        "W1": g("W1"), "b1": g("b1").reshape(H, 1),
        "W2": g("W2"), "b2": g("b2").reshape(H, 1),
        "W3": g("W3"), "b3": g("b3").reshape(A, 1),
        "ident": np.eye(128, dtype=np.float32),
        "ones_row": np.ones((1, 128), np.float32),
    }
    in_maps = []
    for i in range(NCORES):
        m = dict(common)
        xs = x[i * NLOC:(i + 1) * NLOC]
        m["xT_shard"] = np.ascontiguousarray(xs.T)
        m["src_idx"] = per_core[i]["src_idx"]
        m["P"] = per_core[i]["P"]
        in_maps.append(m)

    last_results = run_bass_kernel_spmd(nc, in_maps, list(range(NCORES)),
                                        trace=trace)
    out = np.concatenate([last_results.results[i]["out"] for i in range(NCORES)], 0)
    return np.ascontiguousarray(out)


def kernel(**inputs) -> np.ndarray:
    return run(inputs, trace=False)


# revision 10
# speedup vs baseline: 1.0608x; 1.0608x over previous
"""Distributed Trainium2 Bass kernel for the GAT-Actor (gnn_message_passing).

Strategy (8 NeuronCores, 1-D node partition):
  - nodes sharded contiguously: core i owns rows [i*NLOC, (i+1)*NLOC)
  - edges assigned to the core owning their DESTINATION node
  - stage 1 (f32): h = x_shard @ W (x pre-transposed on host); per node a
    768B f32 table row [h(128) | 1.0 | e_src | pad]; rows are written in two
    pieces (locals < 3200 / >= 3200) and AllGathered piece-wise so the edge
    stage can start after the first collective.
  - stage 2: edges sorted by (dst-chunk, src-piece) into 128-edge blocks;
    dma_gather pulls 768B rows.  The dst-scatter onehot P[e, d] per block is
    HOST-precomputed (0/1 bf16, pad rows zero) and streamed from HBM - no
    on-device onehot builds.  Per block:
      tloc (e_dst per edge) = DVE stt accum of P * broadcast(e_dst row),
      S = ACT copy-scale of P by w (per-partition scalar, f32 out),
      agg[d, 0:130] += S^T [h | 1 | e_src]  (TensorE; col 128 = softmax
      denominator - ones column fused, no separate den matmul).
    Chunk tail: scale rows by 1/den, PE-transpose to feature-major,
    bias+relu -> h0T f32.
  - stage 3 (f32): BN stats via 1KB AllReduce folded into rescaled fc
    weights; fc1/fc2/fc3 on TensorE; row softmax; [NLOC, 32] shards
    concatenated on host.
"""

import os
import sys

for _p in ("/opt/trn_rl_repo", "/root/.axon_site/_ro/trn_rl_repo"):
    if os.path.isdir(_p) and _p not in sys.path:
        sys.path.insert(0, _p)

import numpy as np
import ml_dtypes

from concourse import bass, bacc, tile, mybir
from concourse.bass_utils import run_bass_kernel_spmd

f32 = mybir.dt.float32
bf16 = mybir.dt.bfloat16
fp16 = mybir.dt.float16
i16 = mybir.dt.int16
AF = mybir.ActivationFunctionType
ALU = mybir.AluOpType

NCORES = 8
C = 128                # dst-chunk width
NEG_SLOPE = 0.2
EPS = 1e-5
PL = 3200              # piece boundary in local rows (25 tiles of 128)
G_CH = 2               # chunks per gather group

_cache = {}
last_results = None


# --------------------------------------------------------------------------
# host-side edge preprocessing
# --------------------------------------------------------------------------

def _wrap_idx(idx):
    """int16 index stream -> [128, len/16] wrapped+replicated for dma_gather."""
    idx = np.asarray(idx, np.int16)
    m = idx.shape[0]
    assert m % 16 == 0
    arr = idx.reshape(m // 16, 16).T
    return np.ascontiguousarray(np.tile(arr, (8, 1)))


def _prep_edges(edge_index, N, NLOC):
    """Sort edges per dst-core by (dst-chunk, src-piece); pad each
    (chunk, piece) to 128-edge blocks shared across cores.  Returns the
    per-core index streams + onehot P tiles and the shared block layout.
    """
    src = np.asarray(edge_index[0], np.int64)
    dst = np.asarray(edge_index[1], np.int64)
    NCH = -(-NLOC // C)
    PB = NLOC - PL

    cores = []
    counts = np.zeros((NCORES, NCH, 2), np.int64)
    for i in range(NCORES):
        sel = (dst // NLOC) == i
        s = src[sel]
        d = dst[sel] - i * NLOC
        ch = d // C
        cs = s // NLOC
        loc = s % NLOC
        hf = (loc >= PL).astype(np.int64)
        idx16 = np.where(hf == 0, cs * PL + loc, cs * PB + (loc - PL))
        order = np.lexsort((hf, ch))
        s_i, d_i, ch_i, hf_i = idx16[order], d[order], ch[order], hf[order]
        for c in range(NCH):
            m = ch_i == c
            counts[i, c, 0] = np.count_nonzero(m & (hf_i == 0))
            counts[i, c, 1] = np.count_nonzero(m & (hf_i == 1))
        cores.append((s_i, d_i, ch_i, hf_i))

    NA = [int(-(-counts[:, c, 0].max() // 128)) for c in range(NCH)]
    NB = [int(-(-counts[:, c, 1].max() // 128)) for c in range(NCH)]

    groups = [list(range(g, min(g + G_CH, NCH))) for g in range(0, NCH, G_CH)]

    blk_of = {}
    goff = 0
    ginfo = []
    for chunks in groups:
        nA = sum(NA[c] for c in chunks)
        nB = sum(NB[c] for c in chunks)
        off = goff
        for c in chunks:
            blk_of[(c, 0)] = off
            off += NA[c]
        for c in chunks:
            blk_of[(c, 1)] = off
            off += NB[c]
        ginfo.append((chunks, goff, nA, nB))
        goff += nA + nB
    TOTB = goff
    TOTE = TOTB * 128

    dr = np.arange(C, dtype=np.int64)
    per_core = []
    for i in range(NCORES):
        s_i, d_i, ch_i, hf_i = cores[i]
        src_idx = np.zeros(TOTE, np.int16)
        dst_rel = np.full(TOTE, -1, np.int64)
        ptr = 0
        for c in range(NCH):
            for h in (0, 1):
                cnt = int(counts[i, c, h])
                sl = slice(ptr, ptr + cnt)
                ptr += cnt
                pos = blk_of[(c, h)] * 128
                if cnt:
                    src_idx[pos:pos + cnt] = s_i[sl].astype(np.int16)
                    dst_rel[pos:pos + cnt] = d_i[sl] - c * C
        assert ptr == len(s_i)
        # P tiles: [TOTB*128, 128] bf16, row b*128+e = onehot(dst_rel)
        drel = dst_rel.reshape(TOTB, 128)
        P = (drel[:, :, None] == dr[None, None, :]).astype(np.float16)
        per_core.append({
            "src_idx": _wrap_idx(src_idx),
            "P": np.ascontiguousarray(P.reshape(TOTB * 128, 128)),
        })
    return per_core, NA, NB, ginfo, blk_of, TOTB


# --------------------------------------------------------------------------
# device graph
# --------------------------------------------------------------------------

def _build_nc(N, D, H, A, NLOC, NA, NB, ginfo, blk_of, TOTB, M):
    KD = D // 128
    NT = -(-NLOC // 128)
    NLOCP = NT * 128
    NCH = len(NA)
    PB = NLOC - PL
    RA = NCORES * PL
    RB = NCORES * PB
    ROWW = 256                # fp16 elems per table row (512B)
    MRW = 130                 # meaningful row width: h(128) | 1 | e_src

    nc = bacc.Bacc("TRN2", num_devices=NCORES)

    xT_in = nc.dram_tensor("xT_shard", [D, NLOC], f32, kind="ExternalInput")
    W_in = nc.dram_tensor("W", [D, H], f32, kind="ExternalInput")
    asrcb = nc.dram_tensor("asrc_b", [128, H], f32, kind="ExternalInput")
    adstb = nc.dram_tensor("adst_b", [128, H], f32, kind="ExternalInput")
    bgat = nc.dram_tensor("b_gat", [H, 1], f32, kind="ExternalInput")
    bn0p = nc.dram_tensor("bn0p", [H, 2], f32, kind="ExternalInput")
    bn2p = nc.dram_tensor("bn2p", [H, 2], f32, kind="ExternalInput")
    W1_in = nc.dram_tensor("W1", [H, H], f32, kind="ExternalInput")
    b1_in = nc.dram_tensor("b1", [H, 1], f32, kind="ExternalInput")
    W2_in = nc.dram_tensor("W2", [H, H], f32, kind="ExternalInput")
    b2_in = nc.dram_tensor("b2", [H, 1], f32, kind="ExternalInput")
    W3_in = nc.dram_tensor("W3", [H, A], f32, kind="ExternalInput")
    b3_in = nc.dram_tensor("b3", [A, 1], f32, kind="ExternalInput")
    ident_in = nc.dram_tensor("ident", [128, 128], f32, kind="ExternalInput")
    onesrow_in = nc.dram_tensor("ones_row", [1, 128], f32, kind="ExternalInput")
    srci_in = nc.dram_tensor("src_idx", [128, TOTB * 8], i16, kind="ExternalInput")
    P_in = nc.dram_tensor("P", [TOTB * 128, 128], fp16, kind="ExternalInput")

    out_t = nc.dram_tensor("out", [NLOC, A], f32, kind="ExternalOutput")

    with tile.TileContext(nc) as tc:
        with tc.tile_pool(name="const", bufs=1) as cp, \
             tc.tile_pool(name="dram", bufs=1, space="DRAM") as dram, \
             tc.tile_pool(name="big", bufs=1) as bigp:

            srci_sb = bigp.tile([128, TOTB * 8], i16)
            nc.sync.dma_start(srci_sb[:], srci_in[:])
            W_sb = cp.tile([128, KD, H], f32)
            nc.sync.dma_start(W_sb[:], bass.AP(W_in, 0, [[H, 128], [128 * H, KD], [1, H]]))
            ident = cp.tile([128, 128], f32)
            nc.sync.dma_start(ident[:], ident_in[:])
            asrc_sb = cp.tile([128, H], f32)
            nc.sync.dma_start(asrc_sb[:], asrcb[:])
            adst_sb = cp.tile([128, H], f32)
            nc.sync.dma_start(adst_sb[:], adstb[:])
            bgat_sb = cp.tile([H, 1], f32)
            nc.sync.dma_start(bgat_sb[:], bgat[:])
            bn0_sb = cp.tile([H, 2], f32)
            nc.sync.dma_start(bn0_sb[:], bn0p[:])
            bn2_sb = cp.tile([H, 2], f32)
            nc.sync.dma_start(bn2_sb[:], bn2p[:])
            W1_sb = cp.tile([H, H], f32)
            nc.sync.dma_start(W1_sb[:], W1_in[:])
            b1_sb = cp.tile([H, 1], f32)
            nc.sync.dma_start(b1_sb[:], b1_in[:])
            W2_sb = cp.tile([H, H], f32)
            nc.sync.dma_start(W2_sb[:], W2_in[:])
            b2_sb = cp.tile([H, 1], f32)
            nc.sync.dma_start(b2_sb[:], b2_in[:])
            W3_sb = cp.tile([H, A], f32)
            nc.sync.dma_start(W3_sb[:], W3_in[:])
            b3_sb = cp.tile([A, 1], f32)
            nc.sync.dma_start(b3_sb[:], b3_in[:])
            onesr = cp.tile([1, 128], f32)
            nc.sync.dma_start(onesr[:], onesrow_in[:])

            hlocA = dram.tile([PL, ROWW], fp16)
            hlocB = dram.tile([PB, ROWW], fp16)
            hfullA = dram.tile([RA, ROWW], fp16, addr_space="Shared")
            hfullB = dram.tile([RB, ROWW], fp16, addr_space="Shared")
            bn_in_0 = dram.tile([H, 2], f32)
            bn_out_0 = dram.tile([H, 2], f32, addr_space="Shared")
            bn_in_1 = dram.tile([H, 2], f32)
            bn_out_1 = dram.tile([H, 2], f32, addr_space="Shared")

            edstloc = bigp.tile([128, NT], f32)

            # ================= stage 1: h rows + e_src/e_dst ================
            with tc.tile_pool(name="s1", bufs=3) as s1p, \
                 tc.tile_pool(name="s1ps", bufs=2, space="PSUM") as s1ps:
                for t in range(NT):
                    rows = min(128, NLOC - t * 128)
                    xT_t = s1p.tile([128, KD, 128], f32, tag="xt")
                    for k in range(KD):
                        nc.sync.dma_start(
                            xT_t[:, k, 0:rows],
                            xT_in[k * 128:(k + 1) * 128,
                                  t * 128:t * 128 + rows])
                    h_ps = s1ps.tile([128, H], f32, tag="hps")
                    for k in range(KD):
                        nc.tensor.matmul(h_ps[:], xT_t[:, k, :], W_sb[:, k, :],
                                         start=(k == 0), stop=(k == KD - 1))
                    h_row = s1p.tile([128, MRW], fp16, tag="hrow")
                    nc.vector.tensor_copy(h_row[:, 0:H], h_ps[:])
                    nc.vector.memset(h_row[:, H:H + 1], 1.0)
                    scr = s1p.tile([128, H], f32, tag="scr")
                    e1 = s1p.tile([128, 1], f32, tag="e1")
                    nc.vector.scalar_tensor_tensor(
                        out=scr[:], in0=h_ps[:], scalar=1.0, in1=asrc_sb[:],
                        op0=ALU.mult, op1=ALU.mult,
                        accum_out=e1[:])
                    nc.vector.tensor_copy(h_row[:, MRW - 1:MRW], e1[:])
                    scr2 = s1p.tile([128, H], f32, tag="scr2")
                    nc.vector.scalar_tensor_tensor(
                        out=scr2[:], in0=h_ps[:], scalar=1.0, in1=adst_sb[:],
                        op0=ALU.mult, op1=ALU.mult,
                        accum_out=edstloc[:, t:t + 1])
                    if t < 25:
                        nc.sync.dma_start(
                            bass.AP(hlocA.tensor, t * 128 * ROWW,
                                    [[ROWW, rows], [1, MRW]]),
                            h_row[:rows, :])
                    else:
                        r0 = (t - 25) * 128
                        nc.sync.dma_start(
                            bass.AP(hlocB.tensor, r0 * ROWW,
                                    [[ROWW, rows], [1, MRW]]),
                            h_row[:rows, :])
                    if t == 24:
                        # piece A complete: AllGather it while piece B computes
                        nc.gpsimd.collective_compute(
                            "AllGather", ALU.bypass,
                            replica_groups=[list(range(NCORES))],
                            ins=[hlocA.opt()], outs=[hfullA.opt()])

            nc.gpsimd.collective_compute(
                "AllGather", ALU.bypass, replica_groups=[list(range(NCORES))],
                ins=[hlocB.opt()], outs=[hfullB.opt()])

            # ================= stage 2: edge aggregation ===================
            h0T = bigp.tile([128, NLOCP], f32)
            if NLOC != NLOCP:
                nc.vector.memset(h0T[:, NLOC:NLOCP], 0.0)
            s1cols = bigp.tile([128, NCH], f32)
            s2cols = bigp.tile([128, NCH], f32)
            with tc.tile_pool(name="s2", bufs=2) as s2p, \
                 tc.tile_pool(name="s2s", bufs=4) as s2s, \
                 tc.tile_pool(name="s2ps", bufs=2, space="PSUM") as s2ps:
                for chunks, goff, nAg, nBg in ginfo:
                    nblk = nAg + nBg
                    g_t = s2p.tile([128, nblk, ROWW], fp16, tag="g")
                    if nAg:
                        nc.gpsimd.dma_gather(
                            g_t[:, 0:nAg, :], hfullA[:],
                            srci_sb[:, goff * 8: (goff + nAg) * 8],
                            nAg * 128, nAg * 128, ROWW, single_packet=False)
                    if nBg:
                        nc.gpsimd.dma_gather(
                            g_t[:, nAg:nblk, :], hfullB[:],
                            srci_sb[:, (goff + nAg) * 8: (goff + nblk) * 8],
                            nBg * 128, nBg * 128, ROWW, single_packet=False)
                    P_t = s2p.tile([128, nblk, 128], fp16, tag="P")
                    nc.sync.dma_start(
                        P_t[:],
                        bass.AP(P_in, goff * 128 * 128,
                                [[128, 128], [128 * 128, nblk], [1, 128]]))

                    for c in chunks:
                        na, nb = NA[c], NB[c]
                        nbf = na + nb
                        aoff = blk_of[(c, 0)] - goff
                        boff = blk_of[(c, 1)] - goff
                        Cc = min(C, NLOC - c * C)
                        blist = list(range(aoff, aoff + na)) + \
                                list(range(boff, boff + nb))

                        # e_dst broadcast row for this chunk
                        edT_ps = s2ps.tile([1, 128], f32, tag="edT", bufs=2)
                        nc.tensor.matmul(edT_ps[:], edstloc[:, c:c + 1],
                                         ident[:], start=True, stop=True)
                        edrow = s2p.tile([1, 128], f32, tag="edrow", bufs=2)
                        nc.vector.tensor_copy(edrow[:], edT_ps[:])
                        edB_ps = s2ps.tile([128, 128], f32, tag="edB", bufs=2)
                        nc.tensor.matmul(edB_ps[:], onesr[:], edrow[:],
                                         start=True, stop=True)
                        edb = s2p.tile([128, 128], f32, tag="edb", bufs=2)
                        nc.vector.tensor_copy(edb[:], edB_ps[:])

                        # per-edge e_dst: tloc[e] = sum_d P[e,d] * edb[.,d]
                        tlocv = s2p.tile([128, nbf], f32, tag="tloc", bufs=2)
                        for j, b in enumerate(blist):
                            scrT = s2s.tile([128, C], f32, tag="scrT", bufs=8)
                            nc.vector.scalar_tensor_tensor(
                                out=scrT[:], in0=P_t[:, b, :], scalar=1.0,
                                in1=edb[:], op0=ALU.mult, op1=ALU.mult,
                                accum_out=tlocv[:, j:j + 1])

                        # w = exp(leaky(e_src + e_dst))
                        eps_t = s2p.tile([128, nbf], f32, tag="eps", bufs=2)
                        nc.vector.tensor_tensor(
                            out=eps_t[:, 0:na],
                            in0=g_t[:, aoff:aoff + na, MRW - 1],
                            in1=tlocv[:, 0:na], op=ALU.add)
                        nc.vector.tensor_tensor(
                            out=eps_t[:, na:nbf],
                            in0=g_t[:, boff:boff + nb, MRW - 1],
                            in1=tlocv[:, na:nbf], op=ALU.add)
                        lk = s2p.tile([128, nbf], f32, tag="lk", bufs=2)
                        nc.vector.scalar_tensor_tensor(
                            out=lk[:], in0=eps_t[:], scalar=NEG_SLOPE,
                            in1=eps_t[:], op0=ALU.mult, op1=ALU.max)
                        lk2 = s2p.tile([128, nbf], f32, tag="lk2", bufs=2)
                        nc.vector.scalar_tensor_tensor(
                            out=lk2[:], in0=lk[:], scalar=-M,
                            in1=tlocv[:], op0=ALU.add, op1=ALU.subtract)
                        w_t = s2p.tile([128, nbf], f32, tag="w", bufs=2)
                        nc.scalar.activation(w_t[:], lk2[:], AF.Exp)

                        # agg[d, 0:130] += S^T [h | 1 | e_src],  S = P * w
                        agg_ps = s2ps.tile([128, MRW], f32, tag="agg", bufs=2)
                        for j, b in enumerate(blist):
                            S_b = s2s.tile([128, C], fp16, tag="S", bufs=8)
                            nc.scalar.activation(S_b[:], P_t[:, b, :], AF.Copy,
                                                 scale=w_t[:, j:j + 1])
                            nc.tensor.matmul(agg_ps[:], S_b[:],
                                             g_t[:, b, 0:MRW],
                                             start=(j == 0), stop=(j == nbf - 1))

                        den = s2p.tile([128, 1], f32, tag="den", bufs=2)
                        nc.vector.tensor_scalar(
                            out=den[:], in0=agg_ps[:, H:H + 1],
                            scalar1=1e-16, scalar2=None, op0=ALU.max)
                        rden = s2p.tile([128, 1], f32, tag="rden", bufs=2)
                        nc.vector.reciprocal(rden[:], den[:])
                        h0n = s2p.tile([128, 128], f32, tag="h0n", bufs=2)
                        nc.vector.tensor_scalar(
                            out=h0n[:], in0=agg_ps[:, 0:H],
                            scalar1=rden[:], scalar2=None, op0=ALU.mult)
                        tr_ps = s2ps.tile([128, 128], f32, tag="tr", bufs=2)
                        nc.tensor.transpose(tr_ps[:], h0n[:], ident[:])
                        nc.vector.tensor_scalar(
                            out=h0T[:, c * C: c * C + Cc], in0=tr_ps[:, 0:Cc],
                            scalar1=bgat_sb[:], scalar2=0.0,
                            op0=ALU.add, op1=ALU.max)
                        # incremental BN0 stats for this chunk
                        nc.vector.tensor_reduce(
                            out=s1cols[:, c:c + 1],
                            in_=h0T[:, c * C: c * C + Cc],
                            axis=mybir.AxisListType.X, op=ALU.add)
                        sqv = s2s.tile([128, C], f32, tag="sqv", bufs=4)
                        nc.vector.scalar_tensor_tensor(
                            out=sqv[:, 0:Cc], in0=h0T[:, c * C: c * C + Cc],
                            scalar=1.0, in1=h0T[:, c * C: c * C + Cc],
                            op0=ALU.mult, op1=ALU.mult,
                            accum_out=s2cols[:, c:c + 1])

            # ================= stage 3: BN0 + MLP + softmax ================
            with tc.tile_pool(name="s3", bufs=2) as s3p, \
                 tc.tile_pool(name="s3ps", bufs=2, space="PSUM") as s3ps:

                def bn_fold(hT, k, Wnext_sb, bnext_sb, M, stats=None):
                    s1 = s3p.tile([128, 1], f32, tag="bn1")
                    s2 = s3p.tile([128, 1], f32, tag="bn2t")
                    if stats is not None:
                        nc.vector.tensor_reduce(out=s1[:], in_=stats[0][:],
                                                axis=mybir.AxisListType.X,
                                                op=ALU.add)
                        nc.vector.tensor_reduce(out=s2[:], in_=stats[1][:],
                                                axis=mybir.AxisListType.X,
                                                op=ALU.add)
                    else:
                        nc.vector.tensor_reduce(out=s1[:], in_=hT[:, 0:NLOC],
                                                axis=mybir.AxisListType.X,
                                                op=ALU.add)
                        nsq = -(-NLOC // 512)
                        sqcols = s3p.tile([128, nsq], f32, tag="bnsq" + str(k))
                        for si in range(nsq):
                            s0 = si * 512
                            ln = min(512, NLOC - s0)
                            sq = s3p.tile([128, 512], f32, tag="sqscr", bufs=2)
                            nc.scalar.activation(sq[:, 0:ln], hT[:, s0:s0 + ln],
                                                 AF.Square,
                                                 accum_out=sqcols[:, si:si + 1])
                        nc.vector.tensor_reduce(out=s2[:], in_=sqcols[:],
                                                axis=mybir.AxisListType.X,
                                                op=ALU.add)
                    bnio = s3p.tile([128, 2], f32, tag="bnio")
                    nc.vector.tensor_copy(bnio[:, 0:1], s1[:])
                    nc.vector.tensor_copy(bnio[:, 1:2], s2[:])
                    bn_in_d = bn_in_0 if k == 0 else bn_in_1
                    bn_out_d = bn_out_0 if k == 0 else bn_out_1
                    nc.sync.dma_start(bn_in_d[:], bnio[:])
                    nc.gpsimd.collective_compute(
                        "AllReduce", ALU.add, replica_groups=[list(range(NCORES))],
                        ins=[bn_in_d.opt()], outs=[bn_out_d.opt()])
                    bnst = s3p.tile([128, 2], f32, tag="bnst")
                    nc.sync.dma_start(bnst[:], bn_out_d[:])
                    mu = s3p.tile([128, 1], f32, tag="mu")
                    nc.vector.tensor_scalar(out=mu[:], in0=bnst[:, 0:1],
                                            scalar1=1.0 / N, scalar2=None,
                                            op0=ALU.mult)
                    var = s3p.tile([128, 1], f32, tag="var")
                    nc.vector.tensor_tensor(out=var[:], in0=mu[:], in1=mu[:],
                                            op=ALU.mult)
                    nc.vector.tensor_scalar(out=var[:], in0=var[:], scalar1=-1.0,
                                            scalar2=None, op0=ALU.mult)
                    nc.vector.scalar_tensor_tensor(
                        out=var[:], in0=bnst[:, 1:2], scalar=1.0 / N, in1=var[:],
                        op0=ALU.mult, op1=ALU.add)
                    nc.vector.tensor_scalar(out=var[:], in0=var[:], scalar1=EPS,
                                            scalar2=None, op0=ALU.add)
                    rs = s3p.tile([128, 1], f32, tag="rs")
                    nc.vector.reciprocal(rs[:], var[:])
                    nc.scalar.sqrt(rs[:], rs[:])
                    bnp = bn0_sb if k == 0 else bn2_sb
                    sc = s3p.tile([128, 1], f32, tag="sc")
                    nc.vector.tensor_tensor(out=sc[:], in0=rs[:], in1=bnp[:, 0:1],
                                            op=ALU.mult)
                    u = s3p.tile([128, 1], f32, tag="u")
                    nc.vector.tensor_tensor(out=u[:], in0=mu[:], in1=sc[:],
                                            op=ALU.mult)
                    nc.vector.tensor_sub(u[:], bnp[:, 1:2], u[:])
                    Wp = s3p.tile([128, M], f32, tag="wp" + str(k))
                    nc.vector.tensor_scalar(out=Wp[:], in0=Wnext_sb[:],
                                            scalar1=sc[:], scalar2=None,
                                            op0=ALU.mult)
                    brow_ps = s3ps.tile([1, M], f32, tag="brow", bufs=1)
                    nc.tensor.matmul(brow_ps[:], u[:], Wnext_sb[:],
                                     start=True, stop=True)
                    brow_sb = s3p.tile([1, M], f32, tag="brsb")
                    nc.vector.tensor_copy(brow_sb[:], brow_ps[:])
                    bcol_ps = s3ps.tile([M, 1], f32, tag="bcol", bufs=1)
                    nc.tensor.transpose(bcol_ps[:], brow_sb[:], ident[0:1, 0:1])
                    bp = s3p.tile([M, 1], f32, tag="bp" + str(k))
                    nc.vector.tensor_tensor(out=bp[:], in0=bcol_ps[:],
                                            in1=bnext_sb[:], op=ALU.add)
                    return Wp, bp

                h1T = bigp.tile([128, NLOCP], f32)
                W1p, b1p = bn_fold(h0T, 0, W1_sb, b1_sb, H,
                                   stats=(s1cols, s2cols))
                for s in range(0, NLOC, 512):
                    ln = min(512, NLOC - s)
                    ps = s3ps.tile([128, 512], f32, tag="mlp")
                    nc.tensor.matmul(ps[:, 0:ln], W1p[:], h0T[:, s:s + ln],
                                     start=True, stop=True)
                    nc.scalar.activation(h1T[:, s:s + ln], ps[:, 0:ln], AF.Relu,
                                         bias=b1p[:])
                h2T = h0T  # overwrite in place
                for s in range(0, NLOC, 512):
                    ln = min(512, NLOC - s)
                    ps = s3ps.tile([128, 512], f32, tag="mlp")
                    nc.tensor.matmul(ps[:, 0:ln], W2_sb[:], h1T[:, s:s + ln],
                                     start=True, stop=True)
                    nc.scalar.activation(h2T[:, s:s + ln], ps[:, 0:ln], AF.Relu,
                                         bias=b2_sb[:])
                W3p, b3p = bn_fold(h2T, 1, W3_sb, b3_sb, A)
                actT = bigp.tile([A, NLOCP], f32)
                for s in range(0, NLOC, 512):
                    ln = min(512, NLOC - s)
                    ps = s3ps.tile([A, 512], f32, tag="mlp3")
                    nc.tensor.matmul(ps[:, 0:ln], W3p[:], h2T[:, s:s + ln],
                                     start=True, stop=True)
                    nc.vector.tensor_scalar(out=actT[0:A, s:s + ln],
                                            in0=ps[:, 0:ln],
                                            scalar1=b3p[:], scalar2=None,
                                            op0=ALU.add)
                for t in range(NT):
                    rows = min(128, NLOC - t * 128)
                    a_sb = s3p.tile([128, A], f32, tag="asb")
                    for sub in range(4):
                        nc.vector.transpose(
                            a_sb[32 * sub:32 * sub + 32, 0:A],
                            actT[0:A, t * 128 + 32 * sub: t * 128 + 32 * sub + 32])
                    e_sb = s3p.tile([128, A], f32, tag="esb")
                    nc.scalar.activation(e_sb[:], a_sb[:], AF.Exp)
                    ssum = s3p.tile([128, 1], f32, tag="ssum")
                    nc.vector.tensor_reduce(out=ssum[:], in_=e_sb[:],
                                            axis=mybir.AxisListType.X, op=ALU.add)
                    rsum = s3p.tile([128, 1], f32, tag="rsum")
                    nc.vector.reciprocal(rsum[:], ssum[:])
                    o_sb = s3p.tile([128, A], f32, tag="osb")
                    nc.vector.tensor_scalar(out=o_sb[:], in0=e_sb[:],
                                            scalar1=rsum[:], scalar2=None,
                                            op0=ALU.mult)
                    nc.sync.dma_start(out_t[t * 128: t * 128 + rows, :],
                                      o_sb[:rows, :])

    nc.compile()
    return nc


# --------------------------------------------------------------------------
# public entry point
# --------------------------------------------------------------------------

def run(inputs, trace=False):
    global last_results
    x = np.asarray(inputs["x"], np.float32)
    edge_index = np.asarray(inputs["edge_index"])
    N, D = x.shape
    H = np.asarray(inputs["W"]).shape[1]
    A = np.asarray(inputs["W3"]).shape[1]
    assert N % NCORES == 0
    NLOC = N // NCORES

    per_core, NA, NB, ginfo, blk_of, TOTB = _prep_edges(edge_index, N, NLOC)

    Wf = np.asarray(inputs["W"], np.float32)
    es_max = np.abs(x @ (Wf @ np.asarray(inputs["a_src"], np.float32))).max()
    ed_max = np.abs(x @ (Wf @ np.asarray(inputs["a_dst"], np.float32))).max()
    M = float(max(es_max, ed_max)) + 1.0

    key = (N, D, H, A, NLOC, tuple(NA), tuple(NB), round(M, 3))
    if _cache.get("key") != key:
        _cache["nc"] = _build_nc(N, D, H, A, NLOC, NA, NB, ginfo, blk_of,
                                 TOTB, M)
        _cache["key"] = key
    nc = _cache["nc"]

    g = lambda k: np.ascontiguousarray(np.asarray(inputs[k], np.float32))
    common = {
        "W": g("W"),
        "asrc_b": np.tile(g("a_src")[None, :], (128, 1)),
        "adst_b": np.tile(g("a_dst")[None, :], (128, 1)),
        "b_gat": g("b_gat").reshape(H, 1),
        "bn0p": np.stack([g("g0"), g("beta0")], 1),
        "bn2p": np.stack([g("g2"), g("beta2")], 1),
        "W1": g("W1"), "b1": g("b1").reshape(H, 1),
        "W2": g("W2"), "b2": g("b2").reshape(H, 1),
        "W3": g("W3"), "b3": g("b3").reshape(A, 1),
        "ident": np.eye(128, dtype=np.float32),
        "ones_row": np.ones((1, 128), np.float32),
    }
    in_maps = []
    for i in range(NCORES):
        m = dict(common)
        xs = x[i * NLOC:(i + 1) * NLOC]
        m["xT_shard"] = np.ascontiguousarray(xs.T)
        m["src_idx"] = per_core[i]["src_idx"]
        m["P"] = per_core[i]["P"]
        in_maps.append(m)

    last_results = run_bass_kernel_spmd(nc, in_maps, list(range(NCORES)),
                                        trace=trace)
    out = np.concatenate([last_results.results[i]["out"] for i in range(NCORES)], 0)
    return np.ascontiguousarray(out)


def kernel(**inputs) -> np.ndarray:
    return run(inputs, trace=False)


# revision 13
# speedup vs baseline: 1.0871x; 1.0249x over previous
"""Distributed Trainium2 Bass kernel for the GAT-Actor (gnn_message_passing).

Strategy (8 NeuronCores, 1-D node partition):
  - nodes sharded contiguously: core i owns rows [i*NLOC, (i+1)*NLOC)
  - edges assigned to the core owning their DESTINATION node
  - stage 1 (f32): h = x_shard @ W (x pre-transposed on host); per node a
    768B f32 table row [h(128) | 1.0 | e_src | pad]; rows are written in two
    pieces (locals < 3200 / >= 3200) and AllGathered piece-wise so the edge
    stage can start after the first collective.
  - stage 2: edges sorted by (dst-chunk, src-piece) into 128-edge blocks;
    dma_gather pulls 768B rows.  The dst-scatter onehot P[e, d] per block is
    HOST-precomputed (0/1 bf16, pad rows zero) and streamed from HBM - no
    on-device onehot builds.  Per block:
      tloc (e_dst per edge) = DVE stt accum of P * broadcast(e_dst row),
      S = ACT copy-scale of P by w (per-partition scalar, f32 out),
      agg[d, 0:130] += S^T [h | 1 | e_src]  (TensorE; col 128 = softmax
      denominator - ones column fused, no separate den matmul).
    Chunk tail: scale rows by 1/den, PE-transpose to feature-major,
    bias+relu -> h0T f32.
  - stage 3 (f32): BN stats via 1KB AllReduce folded into rescaled fc
    weights; fc1/fc2/fc3 on TensorE; row softmax; [NLOC, 32] shards
    concatenated on host.
"""

import os
import sys

for _p in ("/opt/trn_rl_repo", "/root/.axon_site/_ro/trn_rl_repo"):
    if os.path.isdir(_p) and _p not in sys.path:
        sys.path.insert(0, _p)

import numpy as np
import ml_dtypes

from concourse import bass, bacc, tile, mybir
from concourse.bass_utils import run_bass_kernel_spmd

f32 = mybir.dt.float32
bf16 = mybir.dt.bfloat16
fp16 = mybir.dt.float16
i16 = mybir.dt.int16
AF = mybir.ActivationFunctionType
ALU = mybir.AluOpType

NCORES = 8
C = 128                # dst-chunk width
NEG_SLOPE = 0.2
EPS = 1e-5
PL = 3200              # piece boundary in local rows (25 tiles of 128)
G_CH = 2               # chunks per gather group

_cache = {}
last_results = None


# --------------------------------------------------------------------------
# host-side edge preprocessing
# --------------------------------------------------------------------------

def _wrap_idx(idx):
    """int16 index stream -> [128, len/16] wrapped+replicated for dma_gather."""
    idx = np.asarray(idx, np.int16)
    m = idx.shape[0]
    assert m % 16 == 0
    arr = idx.reshape(m // 16, 16).T
    return np.ascontiguousarray(np.tile(arr, (8, 1)))


def _prep_edges(edge_index, N, NLOC):
    """Sort edges per dst-core by (dst-chunk, src-piece); pad each
    (chunk, piece) to 128-edge blocks shared across cores.  Returns the
    per-core index streams + onehot P tiles and the shared block layout.
    """
    src = np.asarray(edge_index[0], np.int64)
    dst = np.asarray(edge_index[1], np.int64)
    NCH = -(-NLOC // C)
    PB = NLOC - PL

    cores = []
    counts = np.zeros((NCORES, NCH, 2), np.int64)
    for i in range(NCORES):
        sel = (dst // NLOC) == i
        s = src[sel]
        d = dst[sel] - i * NLOC
        ch = d // C
        cs = s // NLOC
        loc = s % NLOC
        hf = (loc >= PL).astype(np.int64)
        idx16 = np.where(hf == 0, cs * PL + loc, cs * PB + (loc - PL))
        order = np.lexsort((hf, ch))
        s_i, d_i, ch_i, hf_i = idx16[order], d[order], ch[order], hf[order]
        for c in range(NCH):
            m = ch_i == c
            counts[i, c, 0] = np.count_nonzero(m & (hf_i == 0))
            counts[i, c, 1] = np.count_nonzero(m & (hf_i == 1))
        cores.append((s_i, d_i, ch_i, hf_i))

    NA = [int(-(-counts[:, c, 0].max() // 128)) for c in range(NCH)]
    NB = [int(-(-counts[:, c, 1].max() // 128)) for c in range(NCH)]

    groups = [list(range(g, min(g + G_CH, NCH))) for g in range(0, NCH, G_CH)]

    blk_of = {}
    goff = 0
    ginfo = []
    for chunks in groups:
        nA = sum(NA[c] for c in chunks)
        nB = sum(NB[c] for c in chunks)
        off = goff
        for c in chunks:
            blk_of[(c, 0)] = off
            off += NA[c]
        for c in chunks:
            blk_of[(c, 1)] = off
            off += NB[c]
        ginfo.append((chunks, goff, nA, nB))
        goff += nA + nB
    TOTB = goff
    TOTE = TOTB * 128

    dr = np.arange(C, dtype=np.int64)
    per_core = []
    for i in range(NCORES):
        s_i, d_i, ch_i, hf_i = cores[i]
        src_idx = np.zeros(TOTE, np.int16)
        dst_rel = np.full(TOTE, -1, np.int64)
        ptr = 0
        for c in range(NCH):
            for h in (0, 1):
                cnt = int(counts[i, c, h])
                sl = slice(ptr, ptr + cnt)
                ptr += cnt
                pos = blk_of[(c, h)] * 128
                if cnt:
                    src_idx[pos:pos + cnt] = s_i[sl].astype(np.int16)
                    dst_rel[pos:pos + cnt] = d_i[sl] - c * C
        assert ptr == len(s_i)
        # P tiles: [TOTB*128, 128] bf16, row b*128+e = onehot(dst_rel)
        drel = dst_rel.reshape(TOTB, 128)
        P = (drel[:, :, None] == dr[None, None, :]).astype(np.float16)
        per_core.append({
            "src_idx": _wrap_idx(src_idx),
            "P": np.ascontiguousarray(P.reshape(TOTB * 128, 128)),
        })
    return per_core, NA, NB, ginfo, blk_of, TOTB


# --------------------------------------------------------------------------
# device graph
# --------------------------------------------------------------------------

def _build_nc(N, D, H, A, NLOC, NA, NB, ginfo, blk_of, TOTB, M):
    KD = D // 128
    NT = -(-NLOC // 128)
    NLOCP = NT * 128
    NCH = len(NA)
    PB = NLOC - PL
    RA = NCORES * PL
    RB = NCORES * PB
    ROWW = 256                # fp16 elems per table row (512B)
    MRW = 130                 # meaningful row width: h(128) | 1 | e_src

    nc = bacc.Bacc("TRN2", num_devices=NCORES)

    xT_in = nc.dram_tensor("xT_shard", [D, NLOC], fp16, kind="ExternalInput")
    W_in = nc.dram_tensor("W", [D, H], fp16, kind="ExternalInput")
    asrcb = nc.dram_tensor("asrc_b", [128, H], f32, kind="ExternalInput")
    adstb = nc.dram_tensor("adst_b", [128, H], f32, kind="ExternalInput")
    bgat = nc.dram_tensor("b_gat", [H, 1], f32, kind="ExternalInput")
    bn0p = nc.dram_tensor("bn0p", [H, 2], f32, kind="ExternalInput")
    bn2p = nc.dram_tensor("bn2p", [H, 2], f32, kind="ExternalInput")
    W1_in = nc.dram_tensor("W1", [H, H], fp16, kind="ExternalInput")
    b1_in = nc.dram_tensor("b1", [H, 1], f32, kind="ExternalInput")
    W2_in = nc.dram_tensor("W2", [H, H], fp16, kind="ExternalInput")
    b2_in = nc.dram_tensor("b2", [H, 1], f32, kind="ExternalInput")
    W3_in = nc.dram_tensor("W3", [H, A], fp16, kind="ExternalInput")
    b3_in = nc.dram_tensor("b3", [A, 1], f32, kind="ExternalInput")
    ident_in = nc.dram_tensor("ident", [128, 128], f32, kind="ExternalInput")
    onesrow_in = nc.dram_tensor("ones_row", [1, 128], f32, kind="ExternalInput")
    srci_in = nc.dram_tensor("src_idx", [128, TOTB * 8], i16, kind="ExternalInput")
    P_in = nc.dram_tensor("P", [TOTB * 128, 128], fp16, kind="ExternalInput")

    out_t = nc.dram_tensor("out", [NLOC, A], f32, kind="ExternalOutput")

    with tile.TileContext(nc) as tc:
        with tc.tile_pool(name="const", bufs=1) as cp, \
             tc.tile_pool(name="dram", bufs=1, space="DRAM") as dram, \
             tc.tile_pool(name="big", bufs=1) as bigp:

            srci_sb = bigp.tile([128, TOTB * 8], i16)
            nc.sync.dma_start(srci_sb[:], srci_in[:])
            W_sb = cp.tile([128, KD, H], fp16)
            nc.sync.dma_start(W_sb[:], bass.AP(W_in, 0, [[H, 128], [128 * H, KD], [1, H]]))
            ident = cp.tile([128, 128], f32)
            nc.sync.dma_start(ident[:], ident_in[:])
            asrc_sb = cp.tile([128, H], f32)
            nc.sync.dma_start(asrc_sb[:], asrcb[:])
            adst_sb = cp.tile([128, H], f32)
            nc.sync.dma_start(adst_sb[:], adstb[:])
            bgat_sb = cp.tile([H, 1], f32)
            nc.sync.dma_start(bgat_sb[:], bgat[:])
            bn0_sb = cp.tile([H, 2], f32)
            nc.sync.dma_start(bn0_sb[:], bn0p[:])
            bn2_sb = cp.tile([H, 2], f32)
            nc.sync.dma_start(bn2_sb[:], bn2p[:])
            W1_sb = cp.tile([H, H], fp16)
            nc.sync.dma_start(W1_sb[:], W1_in[:])
            b1_sb = cp.tile([H, 1], f32)
            nc.sync.dma_start(b1_sb[:], b1_in[:])
            W2_sb = cp.tile([H, H], fp16)
            nc.sync.dma_start(W2_sb[:], W2_in[:])
            b2_sb = cp.tile([H, 1], f32)
            nc.sync.dma_start(b2_sb[:], b2_in[:])
            W3_sb = cp.tile([H, A], fp16)
            nc.sync.dma_start(W3_sb[:], W3_in[:])
            b3_sb = cp.tile([A, 1], f32)
            nc.sync.dma_start(b3_sb[:], b3_in[:])
            onesr = cp.tile([1, 128], f32)
            nc.sync.dma_start(onesr[:], onesrow_in[:])

            hlocA = dram.tile([PL, ROWW], fp16)
            hlocB = dram.tile([PB, ROWW], fp16)
            hfullA = dram.tile([RA, ROWW], fp16, addr_space="Shared")
            hfullB = dram.tile([RB, ROWW], fp16, addr_space="Shared")
            bn_in_0 = dram.tile([H, 2], f32)
            bn_out_0 = dram.tile([H, 2], f32, addr_space="Shared")
            bn_in_1 = dram.tile([H, 2], f32)
            bn_out_1 = dram.tile([H, 2], f32, addr_space="Shared")

            edstloc = bigp.tile([128, NT], f32)

            # ================= stage 1: h rows + e_src/e_dst ================
            with tc.tile_pool(name="s1", bufs=3) as s1p, \
                 tc.tile_pool(name="s1ps", bufs=2, space="PSUM") as s1ps:
                for t in range(NT):
                    rows = min(128, NLOC - t * 128)
                    xT_t = s1p.tile([128, KD, 128], fp16, tag="xt")
                    for k in range(KD):
                        nc.sync.dma_start(
                            xT_t[:, k, 0:rows],
                            xT_in[k * 128:(k + 1) * 128,
                                  t * 128:t * 128 + rows])
                    h_ps = s1ps.tile([128, H], f32, tag="hps")
                    for k in range(KD):
                        nc.tensor.matmul(h_ps[:], xT_t[:, k, :], W_sb[:, k, :],
                                         start=(k == 0), stop=(k == KD - 1))
                    h_row = s1p.tile([128, MRW], fp16, tag="hrow")
                    nc.vector.tensor_copy(h_row[:, 0:H], h_ps[:])
                    nc.vector.memset(h_row[:, H:H + 1], 1.0)
                    scr = s1p.tile([128, H], f32, tag="scr")
                    e1 = s1p.tile([128, 1], f32, tag="e1")
                    nc.vector.scalar_tensor_tensor(
                        out=scr[:], in0=h_ps[:], scalar=1.0, in1=asrc_sb[:],
                        op0=ALU.mult, op1=ALU.mult,
                        accum_out=e1[:])
                    nc.vector.tensor_copy(h_row[:, MRW - 1:MRW], e1[:])
                    scr2 = s1p.tile([128, H], f32, tag="scr2")
                    nc.vector.scalar_tensor_tensor(
                        out=scr2[:], in0=h_ps[:], scalar=1.0, in1=adst_sb[:],
                        op0=ALU.mult, op1=ALU.mult,
                        accum_out=edstloc[:, t:t + 1])
                    if t < 25:
                        nc.sync.dma_start(
                            bass.AP(hlocA.tensor, t * 128 * ROWW,
                                    [[ROWW, rows], [1, MRW]]),
                            h_row[:rows, :])
                    else:
                        r0 = (t - 25) * 128
                        nc.sync.dma_start(
                            bass.AP(hlocB.tensor, r0 * ROWW,
                                    [[ROWW, rows], [1, MRW]]),
                            h_row[:rows, :])
                    if t == 24:
                        # piece A complete: AllGather it while piece B computes
                        nc.gpsimd.collective_compute(
                            "AllGather", ALU.bypass,
                            replica_groups=[list(range(NCORES))],
                            ins=[hlocA.opt()], outs=[hfullA.opt()])

            nc.gpsimd.collective_compute(
                "AllGather", ALU.bypass, replica_groups=[list(range(NCORES))],
                ins=[hlocB.opt()], outs=[hfullB.opt()])

            # ================= stage 2: edge aggregation ===================
            h0T = bigp.tile([128, NLOCP], fp16)
            if NLOC != NLOCP:
                nc.vector.memset(h0T[:, NLOC:NLOCP], 0.0)
            s1cols = bigp.tile([128, NCH], f32)
            s2cols = bigp.tile([128, NCH], f32)
            with tc.tile_pool(name="s2", bufs=2) as s2p, \
                 tc.tile_pool(name="s2s", bufs=4) as s2s, \
                 tc.tile_pool(name="s2ps", bufs=2, space="PSUM") as s2ps:
                LEAD = 2
                ng = len(ginfo)
                gtiles = {}

                def issue_A(gi):
                    chunks, goff, nAg, nBg = ginfo[gi]
                    nblk = nAg + nBg
                    g_t = s2p.tile([128, nblk, ROWW], fp16, tag="g",
                                   bufs=LEAD + 2)
                    P_t = s2p.tile([128, nblk, 128], fp16, tag="P",
                                   bufs=LEAD + 1)
                    gtiles[gi] = (g_t, P_t)
                    nc.sync.dma_start(
                        P_t[:],
                        bass.AP(P_in, goff * 128 * 128,
                                [[128, 128], [128 * 128, nblk], [1, 128]]))
                    if nAg:
                        nc.gpsimd.dma_gather(
                            g_t[:, 0:nAg, :], hfullA[:],
                            srci_sb[:, goff * 8: (goff + nAg) * 8],
                            nAg * 128, nAg * 128, ROWW, single_packet=False)

                def issue_B(gi):
                    chunks, goff, nAg, nBg = ginfo[gi]
                    nblk = nAg + nBg
                    g_t, _ = gtiles[gi]
                    if nBg:
                        nc.gpsimd.dma_gather(
                            g_t[:, nAg:nblk, :], hfullB[:],
                            srci_sb[:, (goff + nAg) * 8: (goff + nblk) * 8],
                            nBg * 128, nBg * 128, ROWW, single_packet=False)

                for gi in range(min(LEAD + 1, ng)):
                    issue_A(gi)
                for gi, (chunks, goff, nAg, nBg) in enumerate(ginfo):
                    issue_B(gi)
                    if gi + LEAD + 1 < ng:
                        issue_A(gi + LEAD + 1)
                    g_t, P_t = gtiles.pop(gi)

                    for c in chunks:
                        na, nb = NA[c], NB[c]
                        nbf = na + nb
                        aoff = blk_of[(c, 0)] - goff
                        boff = blk_of[(c, 1)] - goff
                        Cc = min(C, NLOC - c * C)
                        blist = list(range(aoff, aoff + na)) + \
                                list(range(boff, boff + nb))

                        # e_dst broadcast row for this chunk
                        edT_ps = s2ps.tile([1, 128], f32, tag="edT", bufs=2)
                        nc.tensor.matmul(edT_ps[:], edstloc[:, c:c + 1],
                                         ident[:], start=True, stop=True)
                        edrow = s2p.tile([1, 128], f32, tag="edrow", bufs=2)
                        nc.vector.tensor_copy(edrow[:], edT_ps[:])
                        edB_ps = s2ps.tile([128, 128], f32, tag="edB", bufs=2)
                        nc.tensor.matmul(edB_ps[:], onesr[:], edrow[:],
                                         start=True, stop=True)
                        edb = s2p.tile([128, 128], f32, tag="edb", bufs=2)
                        nc.vector.tensor_copy(edb[:], edB_ps[:])

                        # per-edge e_dst: tloc[e] = sum_d P[e,d] * edb[.,d]
                        tlocv = s2p.tile([128, nbf], f32, tag="tloc", bufs=2)
                        for j, b in enumerate(blist):
                            scrT = s2s.tile([128, C], f32, tag="scrT", bufs=8)
                            nc.vector.scalar_tensor_tensor(
                                out=scrT[:], in0=P_t[:, b, :], scalar=1.0,
                                in1=edb[:], op0=ALU.mult, op1=ALU.mult,
                                accum_out=tlocv[:, j:j + 1])

                        # w = exp(leaky(e_src + e_dst))
                        eps_t = s2p.tile([128, nbf], f32, tag="eps", bufs=2)
                        nc.vector.tensor_tensor(
                            out=eps_t[:, 0:na],
                            in0=g_t[:, aoff:aoff + na, MRW - 1],
                            in1=tlocv[:, 0:na], op=ALU.add)
                        nc.vector.tensor_tensor(
                            out=eps_t[:, na:nbf],
                            in0=g_t[:, boff:boff + nb, MRW - 1],
                            in1=tlocv[:, na:nbf], op=ALU.add)
                        lk = s2p.tile([128, nbf], f32, tag="lk", bufs=2)
                        nc.vector.scalar_tensor_tensor(
                            out=lk[:], in0=eps_t[:], scalar=NEG_SLOPE,
                            in1=eps_t[:], op0=ALU.mult, op1=ALU.max)
                        lk2 = s2p.tile([128, nbf], f32, tag="lk2", bufs=2)
                        nc.vector.scalar_tensor_tensor(
                            out=lk2[:], in0=lk[:], scalar=-M,
                            in1=tlocv[:], op0=ALU.add, op1=ALU.subtract)
                        w_t = s2p.tile([128, nbf], f32, tag="w", bufs=2)
                        nc.scalar.activation(w_t[:], lk2[:], AF.Exp)

                        # agg[d, 0:130] += S^T [h | 1 | e_src],  S = P * w
                        agg_ps = s2ps.tile([128, MRW], f32, tag="agg", bufs=2)
                        for j, b in enumerate(blist):
                            S_b = s2s.tile([128, C], fp16, tag="S", bufs=8)
                            nc.scalar.activation(S_b[:], P_t[:, b, :], AF.Copy,
                                                 scale=w_t[:, j:j + 1])
                            nc.tensor.matmul(agg_ps[:], S_b[:],
                                             g_t[:, b, 0:MRW],
                                             start=(j == 0), stop=(j == nbf - 1))

                        den = s2p.tile([128, 1], f32, tag="den", bufs=2)
                        nc.vector.tensor_scalar(
                            out=den[:], in0=agg_ps[:, H:H + 1],
                            scalar1=1e-16, scalar2=None, op0=ALU.max)
                        rden = s2p.tile([128, 1], f32, tag="rden", bufs=2)
                        nc.vector.reciprocal(rden[:], den[:])
                        h0n = s2p.tile([128, 128], f32, tag="h0n", bufs=2)
                        nc.vector.tensor_scalar(
                            out=h0n[:], in0=agg_ps[:, 0:H],
                            scalar1=rden[:], scalar2=None, op0=ALU.mult)
                        tr_ps = s2ps.tile([128, 128], f32, tag="tr", bufs=2)
                        nc.tensor.transpose(tr_ps[:], h0n[:], ident[:])
                        nc.vector.tensor_scalar(
                            out=h0T[:, c * C: c * C + Cc], in0=tr_ps[:, 0:Cc],
                            scalar1=bgat_sb[:], scalar2=0.0,
                            op0=ALU.add, op1=ALU.max)
                        # incremental BN0 stats for this chunk
                        nc.vector.tensor_reduce(
                            out=s1cols[:, c:c + 1],
                            in_=h0T[:, c * C: c * C + Cc],
                            axis=mybir.AxisListType.X, op=ALU.add)
                        sqv = s2s.tile([128, C], f32, tag="sqv", bufs=4)
                        nc.vector.scalar_tensor_tensor(
                            out=sqv[:, 0:Cc], in0=h0T[:, c * C: c * C + Cc],
                            scalar=1.0, in1=h0T[:, c * C: c * C + Cc],
                            op0=ALU.mult, op1=ALU.mult,
                            accum_out=s2cols[:, c:c + 1])

            # ================= stage 3: BN0 + MLP + softmax ================
            with tc.tile_pool(name="s3", bufs=2) as s3p, \
                 tc.tile_pool(name="s3ps", bufs=2, space="PSUM") as s3ps:

                def bn_fold(hT, k, Wnext_sb, bnext_sb, M, stats=None):
                    s1 = s3p.tile([128, 1], f32, tag="bn1")
                    s2 = s3p.tile([128, 1], f32, tag="bn2t")
                    if stats is not None:
                        nc.vector.tensor_reduce(out=s1[:], in_=stats[0][:],
                                                axis=mybir.AxisListType.X,
                                                op=ALU.add)
                        nc.vector.tensor_reduce(out=s2[:], in_=stats[1][:],
                                                axis=mybir.AxisListType.X,
                                                op=ALU.add)
                    else:
                        nc.vector.tensor_reduce(out=s1[:], in_=hT[:, 0:NLOC],
                                                axis=mybir.AxisListType.X,
                                                op=ALU.add)
                        nsq = -(-NLOC // 512)
                        sqcols = s3p.tile([128, nsq], f32, tag="bnsq" + str(k))
                        for si in range(nsq):
                            s0 = si * 512
                            ln = min(512, NLOC - s0)
                            sq = s3p.tile([128, 512], f32, tag="sqscr", bufs=2)
                            nc.scalar.activation(sq[:, 0:ln], hT[:, s0:s0 + ln],
                                                 AF.Square,
                                                 accum_out=sqcols[:, si:si + 1])
                        nc.vector.tensor_reduce(out=s2[:], in_=sqcols[:],
                                                axis=mybir.AxisListType.X,
                                                op=ALU.add)
                    bnio = s3p.tile([128, 2], f32, tag="bnio")
                    nc.vector.tensor_copy(bnio[:, 0:1], s1[:])
                    nc.vector.tensor_copy(bnio[:, 1:2], s2[:])
                    bn_in_d = bn_in_0 if k == 0 else bn_in_1
                    bn_out_d = bn_out_0 if k == 0 else bn_out_1
                    nc.sync.dma_start(bn_in_d[:], bnio[:])
                    nc.gpsimd.collective_compute(
                        "AllReduce", ALU.add, replica_groups=[list(range(NCORES))],
                        ins=[bn_in_d.opt()], outs=[bn_out_d.opt()])
                    bnst = s3p.tile([128, 2], f32, tag="bnst")
                    nc.sync.dma_start(bnst[:], bn_out_d[:])
                    mu = s3p.tile([128, 1], f32, tag="mu")
                    nc.vector.tensor_scalar(out=mu[:], in0=bnst[:, 0:1],
                                            scalar1=1.0 / N, scalar2=None,
                                            op0=ALU.mult)
                    var = s3p.tile([128, 1], f32, tag="var")
                    nc.vector.tensor_tensor(out=var[:], in0=mu[:], in1=mu[:],
                                            op=ALU.mult)
                    nc.vector.tensor_scalar(out=var[:], in0=var[:], scalar1=-1.0,
                                            scalar2=None, op0=ALU.mult)
                    nc.vector.scalar_tensor_tensor(
                        out=var[:], in0=bnst[:, 1:2], scalar=1.0 / N, in1=var[:],
                        op0=ALU.mult, op1=ALU.add)
                    nc.vector.tensor_scalar(out=var[:], in0=var[:], scalar1=EPS,
                                            scalar2=None, op0=ALU.add)
                    rs = s3p.tile([128, 1], f32, tag="rs")
                    nc.vector.reciprocal(rs[:], var[:])
                    nc.scalar.sqrt(rs[:], rs[:])
                    bnp = bn0_sb if k == 0 else bn2_sb
                    sc = s3p.tile([128, 1], f32, tag="sc")
                    nc.vector.tensor_tensor(out=sc[:], in0=rs[:], in1=bnp[:, 0:1],
                                            op=ALU.mult)
                    uf = s3p.tile([128, 1], f32, tag="uf")
                    nc.vector.tensor_tensor(out=uf[:], in0=mu[:], in1=sc[:],
                                            op=ALU.mult)
                    nc.vector.tensor_sub(uf[:], bnp[:, 1:2], uf[:])
                    u = s3p.tile([128, 1], fp16, tag="u")
                    nc.vector.tensor_copy(u[:], uf[:])
                    Wp = s3p.tile([128, M], fp16, tag="wp" + str(k))
                    nc.vector.tensor_scalar(out=Wp[:], in0=Wnext_sb[:],
                                            scalar1=sc[:], scalar2=None,
                                            op0=ALU.mult)
                    brow_ps = s3ps.tile([1, M], f32, tag="brow", bufs=1)
                    nc.tensor.matmul(brow_ps[:], u[:], Wnext_sb[:],
                                     start=True, stop=True)
                    brow_sb = s3p.tile([1, M], f32, tag="brsb")
                    nc.vector.tensor_copy(brow_sb[:], brow_ps[:])
                    bcol_ps = s3ps.tile([M, 1], f32, tag="bcol", bufs=1)
                    nc.tensor.transpose(bcol_ps[:], brow_sb[:], ident[0:1, 0:1])
                    bp = s3p.tile([M, 1], f32, tag="bp" + str(k))
                    nc.vector.tensor_tensor(out=bp[:], in0=bcol_ps[:],
                                            in1=bnext_sb[:], op=ALU.add)
                    return Wp, bp

                h1T = bigp.tile([128, NLOCP], fp16)
                W1p, b1p = bn_fold(h0T, 0, W1_sb, b1_sb, H,
                                   stats=(s1cols, s2cols))
                for s in range(0, NLOC, 512):
                    ln = min(512, NLOC - s)
                    ps = s3ps.tile([128, 512], f32, tag="mlp")
                    nc.tensor.matmul(ps[:, 0:ln], W1p[:], h0T[:, s:s + ln],
                                     start=True, stop=True)
                    nc.scalar.activation(h1T[:, s:s + ln], ps[:, 0:ln], AF.Relu,
                                         bias=b1p[:])
                h2T = h0T  # overwrite in place
                for s in range(0, NLOC, 512):
                    ln = min(512, NLOC - s)
                    ps = s3ps.tile([128, 512], f32, tag="mlp")
                    nc.tensor.matmul(ps[:, 0:ln], W2_sb[:], h1T[:, s:s + ln],
                                     start=True, stop=True)
                    nc.scalar.activation(h2T[:, s:s + ln], ps[:, 0:ln], AF.Relu,
                                         bias=b2_sb[:])
                W3p, b3p = bn_fold(h2T, 1, W3_sb, b3_sb, A)
                actT = bigp.tile([A, NLOCP], fp16)
                for s in range(0, NLOC, 512):
                    ln = min(512, NLOC - s)
                    ps = s3ps.tile([A, 512], f32, tag="mlp3")
                    nc.tensor.matmul(ps[:, 0:ln], W3p[:], h2T[:, s:s + ln],
                                     start=True, stop=True)
                    nc.vector.tensor_scalar(out=actT[0:A, s:s + ln],
                                            in0=ps[:, 0:ln],
                                            scalar1=b3p[:], scalar2=None,
                                            op0=ALU.add)
                for t in range(NT):
                    rows = min(128, NLOC - t * 128)
                    a_sb = s3p.tile([128, A], fp16, tag="asb")
                    for sub in range(4):
                        nc.vector.transpose(
                            a_sb[32 * sub:32 * sub + 32, 0:A],
                            actT[0:A, t * 128 + 32 * sub: t * 128 + 32 * sub + 32])
                    e_sb = s3p.tile([128, A], f32, tag="esb")
                    nc.scalar.activation(e_sb[:], a_sb[:], AF.Exp)
                    ssum = s3p.tile([128, 1], f32, tag="ssum")
                    nc.vector.tensor_reduce(out=ssum[:], in_=e_sb[:],
                                            axis=mybir.AxisListType.X, op=ALU.add)
                    rsum = s3p.tile([128, 1], f32, tag="rsum")
                    nc.vector.reciprocal(rsum[:], ssum[:])
                    o_sb = s3p.tile([128, A], f32, tag="osb")
                    nc.vector.tensor_scalar(out=o_sb[:], in0=e_sb[:],
                                            scalar1=rsum[:], scalar2=None,
                                            op0=ALU.mult)
                    nc.sync.dma_start(out_t[t * 128: t * 128 + rows, :],
                                      o_sb[:rows, :])

    nc.compile()
    return nc


# --------------------------------------------------------------------------
# public entry point
# --------------------------------------------------------------------------

def run(inputs, trace=False):
    global last_results
    x = np.asarray(inputs["x"], np.float32)
    edge_index = np.asarray(inputs["edge_index"])
    N, D = x.shape
    H = np.asarray(inputs["W"]).shape[1]
    A = np.asarray(inputs["W3"]).shape[1]
    assert N % NCORES == 0
    NLOC = N // NCORES

    per_core, NA, NB, ginfo, blk_of, TOTB = _prep_edges(edge_index, N, NLOC)

    Wf = np.asarray(inputs["W"], np.float32)
    es_max = np.abs(x @ (Wf @ np.asarray(inputs["a_src"], np.float32))).max()
    ed_max = np.abs(x @ (Wf @ np.asarray(inputs["a_dst"], np.float32))).max()
    M = float(max(es_max, ed_max)) + 1.0

    key = (N, D, H, A, NLOC, tuple(NA), tuple(NB), round(M, 3))
    if _cache.get("key") != key:
        _cache["nc"] = _build_nc(N, D, H, A, NLOC, NA, NB, ginfo, blk_of,
                                 TOTB, M)
        _cache["key"] = key
    nc = _cache["nc"]

    g = lambda k: np.ascontiguousarray(np.asarray(inputs[k], np.float32))
    g16 = lambda k: np.ascontiguousarray(
        np.asarray(inputs[k], np.float32).astype(np.float16))
    common = {
        "W": g16("W"),
        "asrc_b": np.tile(g("a_src")[None, :], (128, 1)),
        "adst_b": np.tile(g("a_dst")[None, :], (128, 1)),
        "b_gat": g("b_gat").reshape(H, 1),
        "bn0p": np.stack([g("g0"), g("beta0")], 1),
        "bn2p": np.stack([g("g2"), g("beta2")], 1),
        "W1": g16("W1"), "b1": g("b1").reshape(H, 1),
        "W2": g16("W2"), "b2": g("b2").reshape(H, 1),
        "W3": g16("W3"), "b3": g("b3").reshape(A, 1),
        "ident": np.eye(128, dtype=np.float32),
        "ones_row": np.ones((1, 128), np.float32),
    }
    in_maps = []
    for i in range(NCORES):
        m = dict(common)
        xs = x[i * NLOC:(i + 1) * NLOC]
        m["xT_shard"] = np.ascontiguousarray(xs.T).astype(np.float16)
        m["src_idx"] = per_core[i]["src_idx"]
        m["P"] = per_core[i]["P"]
        in_maps.append(m)

    last_results = run_bass_kernel_spmd(nc, in_maps, list(range(NCORES)),
                                        trace=trace)
    out = np.concatenate([last_results.results[i]["out"] for i in range(NCORES)], 0)
    return np.ascontiguousarray(out)


def kernel(**inputs) -> np.ndarray:
    return run(inputs, trace=False)
